# revision 55
# baseline (speedup 1.0000x reference)
"""Fused multi-head attention block (qkv proj + softmax(QK^T)V + out proj)
for Trainium2, SPMD across 8 NeuronCores.

Sharding: 8 cores = 2 batches x 4 head-groups (4 heads/core, data parallel on
B, tensor parallel on heads). Each core computes its 4 heads end-to-end plus a
row-parallel slice of the output projection; the 4 partial y's per batch are
summed on the host (with b_proj added once).

v2 dataflow (fp16 operands, fp32 PSUM accumulation):
  - inputs pre-transposed on host: xT [D,T], wqkT02/wqkT13 [D,256] (q/k for
    head-pair 0 resp. 1), wvT [D,256], wpT [256,D]; aux carries a 128x128
    identity (PE transposes) plus a ones column.
  - QKV: q/k produced in [channel, T] layout (fp8 hi/lo split when
    fp8_scores, else fp16); v produced in [T, channel] fp16 layout with a
    ones column per head (so PV emits softmax denominators as column 64).
  - scores transposed: S.T[j,i] = kT.T @ qT per head, two heads in row
    groups 0-63/64-127. With fp8_scores each (j,hh) is ONE DoubleRow
    matmul (k hi/lo as the 2 k-tiles, q_hi broadcast) at 0.5 cycles/row.
  - softmax: P.T = exp(S.T * 0.125) from PSUM via ScalarE. ACT is the
    bottleneck engine (~133us of exp); the schedule exists to keep it fed:
    deadline-driven qkv chains let the first exp start ~5us in, and
    everything else dribbles into the PE slack under the exp stream.
  - PV DIRECT: O[i,d] += pt.T @ v_aug, out [128 i, 65] per (head, i-chunk)
    -> 65-cycle matmuls instead of 512 (cost model charges out free size).
    4 accumulation chains share each of two psum banks (single start=True
    per era zeroes the bank; later chains write-fresh via the pending-zero
    range tracking).
  - norm: batched reciprocal of column 64 + broadcast multiplies per block.
  - transpose O[i,ch] -> at.T[ch,i] via PE identity transposes (128 cy).
  - proj: yT[m,i] = wpT.T @ at.T accumulated over 2 ch-chunks -> fp16 out.
"""

from contextlib import ExitStack

import ml_dtypes
import numpy as np

import concourse.bass as bass
import concourse.mybir as mybir
import concourse.tile as tile
from concourse import bacc
from concourse.bass_utils import run_bass_kernel_spmd

F32 = mybir.dt.float32
F16 = mybir.dt.float16
FP8 = mybir.dt.float8e4
FT = mybir.ActivationFunctionType
OP = mybir.AluOpType
DR = mybir.MatmulPerfMode.DoubleRow

B, D = 2, 1024
H, HD = 16, 64
NCORES = 8
HPC = 4                # heads per core
CH = HPC * HD          # 256 q/k/v channels per core
P = 128
KC = D // P            # 8 contraction chunks for the projections
SCALE = 1.0 / float(np.sqrt(HD))


def build_body(tc, ctx, T, xT, x8h, x8l, wqk02, wqk13, wvT, wv8T, bqk, bv,
               wpT, aux, yT, fp8_scores=False, v_fp8=False, lag=4,
               dumps=None):
    nc = tc.nc
    TI = T // 512          # 512-wide i (query) chunks
    TJ = T // P            # 128-wide j (key) chunks
    MC = D // P            # 8 output-row chunks

    xT_r = xT.ap().rearrange("(kc p) t -> p kc t", p=P)
    x8h_r = x8h.ap().rearrange("(kc p) t -> p kc t", p=P)
    x8l_r = x8l.ap().rearrange("(kc p) t -> p kc t", p=P)
    wv8_r = wv8T.ap().rearrange("(kc p) m -> p kc m", p=P)
    w02_r = wqk02.ap().rearrange("(kc p) (c m) -> p kc c m", p=P, m=P)
    w13_r = wqk13.ap().rearrange("(kc p) (c m) -> p kc c m", p=P, m=P)
    wv_r = wvT.ap().rearrange("(kc p) m -> p kc m", p=P)
    wp_r = wpT.ap().rearrange("(cc p) m -> p cc m", p=P)
    yT_r = yT.ap().rearrange("(mc p) t -> p mc t", p=P)

    const = ctx.enter_context(tc.tile_pool(name="const", bufs=1))
    qkvp = ctx.enter_context(tc.tile_pool(name="qkvp", bufs=1))
    outp = ctx.enter_context(tc.tile_pool(name="outp", bufs=1))
    # PSUM budget (8 banks): st ring 2x2 banks | po halves 2x1 | psY 2x1
    psS = ctx.enter_context(tc.tile_pool(name="psS", bufs=2, space="PSUM"))
    psO = ctx.enter_context(tc.tile_pool(name="psO", bufs=1, space="PSUM"))
    psY = ctx.enter_context(tc.tile_pool(name="psY", bufs=2, space="PSUM"))

    # ---- constants (gpsimd SWDGE queue, in need order) ----
    bqk_sb = const.tile([P, 4], F32)
    nc.gpsimd.dma_start(out=bqk_sb,
                        in_=bqk.ap().rearrange("(c p) -> p c", p=P))
    aux_sb = const.tile([P, P + 1], F16)  # [:, :128] identity, [:, 128] ones
    nc.gpsimd.dma_start(out=aux_sb, in_=aux.ap())
    ident = aux_sb[:, 0:P]
    bv_sb = const.tile([1, CH], F32)
    nc.gpsimd.dma_start(out=bv_sb, in_=bv.ap())
    # touch Exp once so ACT's table set loads during the input DMA instead of
    # stalling the first real softmax exp mid-pipeline (~2.7us)
    warm = const.tile([1, 1], F32)
    nc.scalar.activation(out=warm, in_=bqk_sb[0:1, 0:1], func=FT.Exp)
    bvb = const.tile([P, CH], F32)
    nc.gpsimd.partition_broadcast(out_ap=bvb, in_ap=bv_sb)

    # ---- persistent activations ----
    if fp8_scores:
        q8_sb = qkvp.tile([P, 2, T], FP8)          # q_hi   [ch, hp, T]
        k8_sb = qkvp.tile([P, 2, 2, T], FP8)       # k hi/lo [ch, hp, hl, T]
    else:
        qk_sb = qkvp.tile([P, 4, T], F16)          # q0,q1,k0,k1 [ch, T]
    v_sb = qkvp.tile([P, TJ, HPC, 65], F16)        # v_aug [T, h, d + ones]
    at_sb = outp.tile([P, 2, T], F16)              # attn_outT [ch, hp, T]

    # ones column per head: the PV matmul then emits softmax denominators
    # as output column 64 (out = in*0 + 1; aux is a clean initialized tile)
    nc.scalar.activation(
        out=v_sb.rearrange("p j h u -> p (j h) u")[:, :, 64],
        in_=aux_sb[:, 0:TJ * HPC],
        func=FT.Copy,
        bias=1.0,
        scale=0.0,
    )

    # ---- input DMA ----
    # Spread across the three DGE queues so the first qk chains can finish
    # ~5us in: xT's first i-block chunks split ACT/SP, per kc; head-pair-0
    # weights on SP; everything deferred on the gpsimd SWDGE queue.
    xw = ctx.enter_context(tc.tile_pool(name="xw", bufs=1))
    xT_sb = xw.tile([P, KC, T], F16)
    w02_sb = xw.tile([P, KC, 2, P], F16)
    w13_sb = xw.tile([P, KC, 2, P], F16)
    wv_sb = xw.tile([P, KC, CH], F16)
    for g in range(KC // 2):
        q = nc.scalar if g % 2 == 0 else nc.sync
        q.dma_start(out=xT_sb[:, 2 * g:2 * g + 2, 0:512],
                    in_=xT_r[:, 2 * g:2 * g + 2, 0:512])
        q2 = nc.sync if g % 2 == 0 else nc.scalar
        q2.dma_start(out=w02_sb[:, 2 * g:2 * g + 2, :, :],
                     in_=w02_r[:, 2 * g:2 * g + 2, :, :])
    if v_fp8:
        x8_sb = xw.tile([P, KC, 2, T], FP8)
        wv8_sb = xw.tile([P, KC, CH], FP8)
        nc.gpsimd.dma_start(out=wv8_sb, in_=wv8_r)
        half = T // 2
        for h in range(2):
            sp = slice(h * half, (h + 1) * half)
            nc.gpsimd.dma_start(out=x8_sb[:, :, 0, sp], in_=x8h_r[:, :, sp])
            nc.gpsimd.dma_start(out=x8_sb[:, :, 1, sp], in_=x8l_r[:, :, sp])
    else:
        for kc in range(KC):
            nc.gpsimd.dma_start(out=wv_sb[:, kc, :], in_=wv_r[:, kc, :])
    if T > 512:
        for g in range(KC // 2):
            q = nc.scalar if g % 2 == 0 else nc.sync
            q.dma_start(out=xT_sb[:, 2 * g:2 * g + 2, 512:T],
                        in_=xT_r[:, 2 * g:2 * g + 2, 512:T])
    for kc in range(KC):
        nc.gpsimd.dma_start(out=w13_sb[:, kc, :, :], in_=w13_r[:, kc, :, :])
    wp_sb = const.tile([P, 2, D], F16)
    for cc in range(2):
        nc.gpsimd.dma_start(out=wp_sb[:, cc, :], in_=wp_r[:, cc, :])

    nps = 0

    def qkv_ps(shape=(P, 512)):
        # the two upfront chains may use an st slot (no scores yet); all
        # deferred chains stay on the py ring so scores staging never waits
        # on a dribbled qkv chain
        nonlocal nps
        nps += 1
        if nps == 1:
            return psS.tile(list(shape), F32, name="ps", tag="st")
        return psY.tile(list(shape), F32, name="ps", tag="py")

    def qk_epilogue(cc, ic, ps):
        hp = cc % 2
        span = slice(ic * 512, (ic + 1) * 512)
        if not fp8_scores:
            nc.vector.tensor_scalar_add(
                out=qk_sb[:, cc, span], in0=ps, scalar1=bqk_sb[:, cc:cc + 1])
        elif cc < 2:   # q: hi only
            nc.vector.tensor_scalar_add(
                out=q8_sb[:, hp, span], in0=ps, scalar1=bqk_sb[:, cc:cc + 1])
        else:          # k: hi + lo
            nc.vector.tensor_scalar_add(
                out=k8_sb[:, hp, 0, span], in0=ps,
                scalar1=bqk_sb[:, cc:cc + 1])
            nc.vector.scalar_tensor_tensor(
                out=k8_sb[:, hp, 1, span], in0=ps,
                scalar=bqk_sb[:, cc:cc + 1], in1=k8_sb[:, hp, 0, span],
                op0=OP.add, op1=OP.subtract)

    def w_of(cc, kc):
        t = w02_sb if cc in (0, 2) else w13_sb
        return t[:, kc, cc // 2, :]

    def qk_chain(cc, ic):
        def f():
            ps = qkv_ps()
            for kc in range(KC):
                nc.tensor.matmul(
                    ps,
                    lhsT=w_of(cc, kc),
                    rhs=xT_sb[:, kc, ic * 512:(ic + 1) * 512],
                    start=(kc == 0),
                    stop=(kc == KC - 1),
                )
            qk_epilogue(cc, ic, ps)
        return f

    chain_state = {}

    def qk_half(cc, ic, g):
        # 4-matmul half-chain: fits under one exp (853ns < 1038ns) so a
        # popped piece never starves the ACT stream
        def f():
            key = ("qk", cc, ic)
            if g == 0:
                chain_state[key] = qkv_ps()
            ps = chain_state[key]
            for kc in range(4 * g, 4 * g + 4):
                nc.tensor.matmul(
                    ps,
                    lhsT=w_of(cc, kc),
                    rhs=xT_sb[:, kc, ic * 512:(ic + 1) * 512],
                    start=(kc == 0),
                    stop=(kc == KC - 1),
                )
            if g == 1:
                qk_epilogue(cc, ic, chain_state.pop(key))
        return f

    def v_half(jc, g):
        def f():
            key = ("v", jc)
            if g == 0:
                chain_state[key] = qkv_ps((P, CH))
            ps = chain_state[key]
            for kc in range(4 * g, 4 * g + 4):
                if v_fp8:
                    nc.tensor.matmul(
                        ps,
                        lhsT=x8_sb[:, kc, :, jc * P:(jc + 1) * P],
                        rhs=wv8_sb[:, kc, :].unsqueeze(1)
                            .broadcast_to([P, 2, CH]),
                        start=(kc == 0),
                        stop=(kc == KC - 1),
                        perf_mode=DR,
                    )
                else:
                    nc.tensor.matmul(
                        ps,
                        lhsT=xT_sb[:, kc, jc * P:(jc + 1) * P],
                        rhs=wv_sb[:, kc, :],
                        start=(kc == 0),
                        stop=(kc == KC - 1),
                    )
            if g == 1:
                nc.vector.tensor_tensor(
                    out=v_sb[:, jc, :, 0:HD],
                    in0=ps.rearrange("p (h u) -> p h u", u=HD),
                    in1=bvb.rearrange("p (h u) -> p h u", u=HD),
                    op=OP.add,
                )
                chain_state.pop(key)
        return f

    def v_chain(jc):
        def f():
            ps = qkv_ps((P, CH))
            for kc in range(KC):
                if v_fp8:
                    nc.tensor.matmul(
                        ps,
                        lhsT=x8_sb[:, kc, :, jc * P:(jc + 1) * P],
                        rhs=wv8_sb[:, kc, :].unsqueeze(1)
                            .broadcast_to([P, 2, CH]),
                        start=(kc == 0),
                        stop=(kc == KC - 1),
                        perf_mode=DR,
                    )
                else:
                    nc.tensor.matmul(
                        ps,
                        lhsT=xT_sb[:, kc, jc * P:(jc + 1) * P],
                        rhs=wv_sb[:, kc, :],
                        start=(kc == 0),
                        stop=(kc == KC - 1),
                    )
            nc.vector.tensor_tensor(
                out=v_sb[:, jc, :, 0:HD],
                in0=ps.rearrange("p (h u) -> p h u", u=HD),
                in1=bvb.rearrange("p (h u) -> p h u", u=HD),
                op=OP.add,
            )
        return f

    # cc 1,3 chains as per-matmul closures (finer dribble granularity)
    qkv_state = {}

    def qk13_piece(cc, ic, kc):
        def f():
            key = (cc, ic)
            if kc == 0:
                qkv_state[key] = qkv_ps()
            ps = qkv_state[key]
            nc.tensor.matmul(
                ps,
                lhsT=w_of(cc, kc),
                rhs=xT_sb[:, kc, ic * 512:(ic + 1) * 512],
                start=(kc == 0),
                stop=(kc == KC - 1),
            )
            if kc == KC - 1:
                qk_epilogue(cc, ic, qkv_state.pop(key))
        return f

    # ---- phase 2: attention + out-proj, one flat software pipeline ------
    work = ctx.enter_context(tc.tile_pool(name="work", bufs=3))
    LAG = lag
    blocks = [(ic, hp) for hp in range(2) for ic in range(TI)]
    steps = [(bi, jj) for bi in range(len(blocks)) for jj in range(TJ)]
    NSTEP = len(steps)
    pt_of = {}
    po_of = {}
    q_proj = []
    proj_state = {}

    # deadline-sorted work queue: (deadline_step, seq, closure)
    sched = []
    seqn = 0

    def add(deadline, closure):
        nonlocal seqn
        sched.append((deadline, seqn, closure))
        seqn += 1

    def upfront_pair():
        ps0 = qkv_ps()
        ps2 = qkv_ps()
        for kc in range(KC):
            for cc, ps in ((0, ps0), (2, ps2)):
                nc.tensor.matmul(
                    ps,
                    lhsT=w_of(cc, kc),
                    rhs=xT_sb[:, kc, 0:512],
                    start=(kc == 0),
                    stop=(kc == KC - 1),
                )
        qk_epilogue(0, 0, ps0)
        qk_epilogue(2, 0, ps2)

    add(-1, upfront_pair)
    for ic in range(1, TI):
        for g in range(2):
            add(4 * ic - 2 + g, qk_half(2, ic, g))   # k(ic) by scores j=4*ic
    for ic in range(1, TI):
        for g in range(2):
            add(TJ * ic - 7 + g, qk_half(0, ic, g))  # q(ic) by block ic
    for jc in range(TJ):
        d = max(0, LAG + 2 * jc - 4)
        add(max(0, d - 1), v_half(jc, 0))
        add(d, v_half(jc, 1))  # before PV(b0, jc)
    # cc1 (q hp1) chain ic needed by block 4+ic start; cc3 (k hp1) chain ic
    # needed by scores(b4, j=4*ic). Deadlines put each 8-piece chain in the
    # 8 steps before its need, so the tail chains dribble into the hp1
    # blocks' PE slack instead of bunching at the transition.
    for ic in range(TI):
        end1 = TJ * (TI + ic) - 6        # block TI+ic scores start
        end3 = TJ * TI + 4 * ic - 4      # scores(b_TI, j=4*ic)
        for kc in range(KC):
            add(end1 - (KC - 1 - kc), qk13_piece(1, ic, kc))
            add(end3 - (KC - 1 - kc), qk13_piece(3, ic, kc))
    sched.sort()

    def emit_scores_exp(bi, jj):
        ic, hp = blocks[bi]
        st = psS.tile([P, 2, 512], F32, name="st", tag="st")
        for hh in range(2):
            sl = slice(hh * 64, (hh + 1) * 64)
            if fp8_scores:
                nc.tensor.matmul(
                    st[:, hh, :],
                    lhsT=k8_sb[sl, hp, :, jj * P:(jj + 1) * P],
                    rhs=q8_sb[sl, hp, ic * 512:(ic + 1) * 512]
                        .unsqueeze(1).broadcast_to([64, 2, 512]),
                    start=True, stop=True, perf_mode=DR,
                )
            else:
                nc.tensor.matmul(
                    st[:, hh, :],
                    lhsT=qk_sb[sl, 2 + hp, jj * P:(jj + 1) * P],
                    rhs=qk_sb[sl, hp, ic * 512:(ic + 1) * 512],
                    start=True, stop=True,
                )
        # ring must cover the max exp->PV distance: block 0's stretched
        # pacing consumes pt(b0, TJ-1) up to TJ+LAG+3 steps after issue
        pt = work.tile([P, 2, 512], F16, name="pt", bufs=TJ + LAG + 4)
        nc.scalar.activation(out=pt, in_=st, func=FT.Exp, scale=SCALE)
        pt_of[(bi, jj)] = pt

    def emit_pv(bi, jj):
        ic, hp = blocks[bi]
        if jj == 0:
            # two 1-bank tiles, 4 accumulation chains each (single start=True
            # per era zeroes the bank; later chains write-fresh into the
            # pending-zero range). Chain c = isub*2+hh -> (half, cc);
            # output col 64 = softmax denominator via the v ones column.
            po_of[bi] = [psO.tile([P, 4, 65], F32, name=f"po{h}", tag=f"po{h}")
                         for h in range(2)]
        po = po_of[bi]
        pt = pt_of.pop((bi, jj))
        for isub in range(4):
            for hh in range(2):
                half, cc = divmod(isub * 2 + hh, 4)
                nc.tensor.matmul(
                    po[half][:, cc, :],
                    lhsT=pt[:, hh, isub * P:(isub + 1) * P],
                    rhs=v_sb[:, jj, hp * 2 + hh, :],
                    start=(jj == 0 and cc == 0),
                    stop=(jj == TJ - 1 and cc == 3),
                    skip_group_check=True)

    def emit_norm(bi):
        ic, hp = blocks[bi]
        po = po_of.pop(bi)
        den_sb = work.tile([P, 2, 4], F32, name="den_sb", bufs=2)
        for half in range(2):
            nc.vector.tensor_copy(out=den_sb[:, half, :],
                                  in_=po[half][:, :, 64])
        rr = work.tile([P, 2, 4], F32, name="rr", bufs=2)
        nc.vector.reciprocal_approx_fast(out=rr, in_=den_sb)
        o_sb = work.tile([P, 4, 2, HD], F16, name="o_sb", bufs=2)
        for half in range(2):
            nc.vector.tensor_tensor(
                out=o_sb[:, half * 2:(half + 1) * 2, :, :],
                in0=po[half][:, :, 0:64].rearrange(
                    "p (i h) d -> p i h d", h=2),
                in1=rr[:, half, :].rearrange("p (i h) -> p i h", h=2)
                      .unsqueeze(3).broadcast_to([P, 2, 2, HD]),
                op=OP.mult,
            )
        last = bi == len(blocks) - 1
        for isub in range(4):
            at_ps = psO.tile([P, P], F16, name="at_ps", tag=f"po{isub % 2}")
            nc.tensor.transpose(at_ps, o_sb[:, isub, :, :], ident)
            span = slice(ic * 512 + isub * P, ic * 512 + (isub + 1) * P)
            if last and isub % 2 == 0:
                # tail: split the copies across ACT (idle after the final
                # exp) and DVE so neither serializes the drain
                nc.scalar.copy(out=at_sb[:, hp, span], in_=at_ps)
            else:
                nc.vector.tensor_copy(out=at_sb[:, hp, span], in_=at_ps)

    def defer_proj(ic):
        def mk(mc, cc):
            def f():
                if cc == 0:
                    proj_state[(ic, mc)] = psY.tile(
                        [P, 512], F32, name="py", tag="py")
                py = proj_state[(ic, mc)]
                nc.tensor.matmul(
                    py,
                    lhsT=wp_sb[:, cc, mc * P:(mc + 1) * P],
                    rhs=at_sb[:, cc, ic * 512:(ic + 1) * 512],
                    start=(cc == 0),
                    stop=(cc == 1),
                )
                if cc == 1:
                    py = proj_state.pop((ic, mc))
                    yt = work.tile([P, 512], F16, name="yt", bufs=6)
                    if ic == TI - 1 and mc % 2 == 0:
                        nc.scalar.copy(out=yt, in_=py)
                    else:
                        nc.vector.tensor_copy(out=yt, in_=py)
                    nc.sync.dma_start(
                        out=yT_r[:, mc, ic * 512:(ic + 1) * 512], in_=yt)
            return f

        q_proj.extend(mk(mc, cc) for mc in range(MC) for cc in range(2))

    # PV emission pacing, as an explicit per-position allowance (linear
    # cursor): first two j's of each block wait 2 extra steps so the
    # previous block's norm can drain before the po era restarts; block 0
    # trickles at half rate because it also carries the 16 v chains.
    def pv_allow(n):
        bi, jj = divmod(n, TJ)
        base = TJ * bi + LAG
        if bi == 0:
            return base + (2 if jj < 2 else 2 * jj)
        return base + (2 if jj < 2 else (3 if jj < 4 else jj))

    pv_done = 0

    def run_pv_to(idx):
        nonlocal pv_done
        while pv_done < NSTEP and pv_allow(pv_done) <= idx:
            bi, jj = steps[pv_done]
            emit_pv(bi, jj)
            pv_done += 1
            if jj == TJ - 1:
                emit_norm(bi)
                ic, hp = blocks[bi]
                if hp == 1:
                    defer_proj(ic)

    while sched and sched[0][0] < 0:
        sched.pop(0)[2]()
    for idx in range(NSTEP + LAG + 2):
        if idx < NSTEP:
            bi, jj = steps[idx]
            emit_scores_exp(bi, jj)
        while sched and sched[0][0] <= idx:
            sched.pop(0)[2]()
        run_pv_to(idx)
        budget = 1 if idx < NSTEP // 2 else (2 if idx < NSTEP else 6)
        for _ in range(budget):
            if sched:
                sched.pop(0)[2]()
            elif q_proj:
                q_proj.pop(0)()
    while q_proj:
        q_proj.pop(0)()

    if dumps is not None:
        if fp8_scores:
            nc.sync.dma_start(out=dumps["qk"].ap(),
                              in_=k8_sb.bitcast(mybir.dt.uint8))
        else:
            nc.sync.dma_start(out=dumps["qk"].ap(), in_=qk_sb.bitcast(F16))
        nc.sync.dma_start(out=dumps["v"].ap(), in_=v_sb.bitcast(F16))
        nc.sync.dma_start(out=dumps["at"].ap(), in_=at_sb.bitcast(F16))


def build_nc(T, dump=False, **kw):
    nc = bacc.Bacc("TRN2", target_bir_lowering=False, debug=False)
    xT = nc.dram_tensor("xT", [D, T], F16, kind="ExternalInput")
    x8h = nc.dram_tensor("x8h", [D, T], FP8, kind="ExternalInput")
    x8l = nc.dram_tensor("x8l", [D, T], FP8, kind="ExternalInput")
    wv8T = nc.dram_tensor("wv8T", [D, CH], FP8, kind="ExternalInput")
    wqk02 = nc.dram_tensor("wqk02", [D, 2 * P], F16, kind="ExternalInput")
    wqk13 = nc.dram_tensor("wqk13", [D, 2 * P], F16, kind="ExternalInput")
    wvT = nc.dram_tensor("wvT", [D, CH], F16, kind="ExternalInput")
    bqk = nc.dram_tensor("bqk", [2 * CH], F32, kind="ExternalInput")
    bv = nc.dram_tensor("bv", [1, CH], F32, kind="ExternalInput")
    wpT = nc.dram_tensor("wpT", [CH, D], F16, kind="ExternalInput")
    aux = nc.dram_tensor("aux", [P, P + 1], F16, kind="ExternalInput")
    yT = nc.dram_tensor("yT", [D, T], F16, kind="ExternalOutput")
    dumps = None
    if dump:
        fp8 = kw.get("fp8_scores", False)
        qk_shape = [P, 2, 2, T] if fp8 else [P, 4, T]
        qk_dt = mybir.dt.uint8 if fp8 else F16
        dumps = {
            "qk": nc.dram_tensor("d_qk", qk_shape, qk_dt,
                                 kind="ExternalOutput"),
            "v": nc.dram_tensor("d_v", [P, T // P, HPC, 65], F16,
                                kind="ExternalOutput"),
            "at": nc.dram_tensor("d_at", [P, 2, T], F16,
                                 kind="ExternalOutput"),
        }
    with tile.TileContext(nc) as tc, ExitStack() as ctx:
        build_body(tc, ctx, T, xT, x8h, x8l, wqk02, wqk13, wvT, wv8T, bqk,
                   bv, wpT, aux, yT, dumps=dumps, **kw)
    nc.compile()
    return nc


def make_in_maps(x, w_attn, b_attn, w_proj):
    x = np.ascontiguousarray(np.asarray(x, dtype=np.float32))
    w_attn = np.asarray(w_attn, dtype=np.float32)
    b_attn = np.asarray(b_attn, dtype=np.float32)
    w_proj = np.asarray(w_proj, dtype=np.float32)
    aux = np.zeros((P, P + 1), np.float16)
    aux[:, :P] = np.eye(P, dtype=np.float16)
    aux[:, P] = 1.0
    in_maps = []
    for c in range(NCORES):
        b, g = divmod(c, 4)
        sl = slice(g * CH, (g + 1) * CH)
        wq, wk, wv = w_attn[0 * D:][sl], w_attn[1 * D:][sl], w_attn[2 * D:][sl]
        # bias order: cc0=q hp0, cc1=q hp1, cc2=k hp0, cc3=k hp1
        bq, bk = b_attn[0 * D:][sl], b_attn[1 * D:][sl]
        # wqk02: [q hp0 | k hp0] columns; wqk13: [q hp1 | k hp1]
        xt = x[b].T
        x8hi = xt.astype(ml_dtypes.float8_e4m3)
        x8lo = (xt - x8hi.astype(np.float32)).astype(ml_dtypes.float8_e4m3)
        in_maps.append({
            "xT": np.ascontiguousarray(xt.astype(np.float16)),
            "x8h": np.ascontiguousarray(x8hi),
            "x8l": np.ascontiguousarray(x8lo),
            "wv8T": np.ascontiguousarray(wv.T.astype(ml_dtypes.float8_e4m3)),
            "wqk02": np.ascontiguousarray(np.concatenate(
                [wq[0:P], wk[0:P]], 0).T.astype(np.float16)),
            "wqk13": np.ascontiguousarray(np.concatenate(
                [wq[P:2 * P], wk[P:2 * P]], 0).T.astype(np.float16)),
            "wvT": np.ascontiguousarray(wv.T.astype(np.float16)),
            "bqk": np.ascontiguousarray(np.concatenate([bq, bk])),
            "bv": np.ascontiguousarray(b_attn[2 * D:][sl][None, :]),
            "wpT": np.ascontiguousarray(w_proj[:, sl].T.astype(np.float16)),
            "aux": aux,
        })
    return in_maps


# default variant configuration
FP8_SCORES = False
LAG_DEFAULT = 28

_NC_CACHE = {}


def _get_nc(T):
    key = (T, FP8_SCORES, LAG_DEFAULT)
    if key not in _NC_CACHE:
        _NC_CACHE[key] = build_nc(T, fp8_scores=FP8_SCORES, lag=LAG_DEFAULT)
    return _NC_CACHE[key]


def run(x, w_attn, b_attn, w_proj, b_proj, trace=False, **hw_kwargs):
    T = np.asarray(x).shape[1]
    nc = _get_nc(T)
    in_maps = make_in_maps(x, w_attn, b_attn, w_proj)
    res = run_bass_kernel_spmd(
        nc, in_maps, core_ids=list(range(NCORES)), trace=trace, **hw_kwargs
    )
    y = np.zeros((B, T, D), dtype=np.float32)
    for c in range(NCORES):
        y[c // 4] += np.asarray(res.results[c]["yT"], dtype=np.float32).T
    y += np.asarray(b_proj, dtype=np.float32)
    return y, res


def kernel(x, w_attn, b_attn, w_proj, b_proj):
    y, _ = run(x, w_attn, b_attn, w_proj, b_proj)
    return y
